# revision 1
# baseline (speedup 1.0000x reference)
"""Trainium2 Bass kernel for nn_AttentionSup (dense transformer attention block).

Computation (see reference):
  qkv = x @ W_qkv; per-head attention softmax(q k^T / sqrt(d)) v;
  domain-gate (tiny MLP + softmax over heads) multiplies the attention
  output per (batch, head, dim); out = gated @ W_out + b_out.

Sharding over 8 NeuronCores: (batch b in 0..3) x (head-group g in 0..1),
4 heads per core — data-parallel over batch, tensor-parallel over heads.
Each core computes a partial output [2048, 512] for its batch from its 4
heads; the host sums the two head-group partials per batch and adds b_out
(the "all-reduce after to_out", done on host since partials per batch live
on exactly 2 cores).

The tiny domain-gate MLP ([4x4] @ [4x32] @ [32x512] per batch) is computed
on the host and folded into the V projection weights (it scales O columns,
i.e. W_v columns). Softmax uses the unnormalized-exp + ones-column trick:
V_ext = [V | 1], so PV matmul also produces row sums; normalization is a
reciprocal broadcast multiply. exp skips max-subtraction (scores ~N(0,1),
max ~5 — no overflow risk in fp32).

All matmuls run in fp32r (single-pass reduced-precision fp32, ~4x faster
than fp32's two half-speed passes; measured end-to-end rel err 3.4e-4).
fp32 PSUM accumulation throughout.

Performance notes (from NTFF traces): the attention phase is paced by the
ScalarE exp stream (~1.3us per [128,1024] tile incl. semaphores). The
tensor engine must stay slightly busier than ScalarE or the PE HAM clock
gate drops it to 1.2 GHz for most of the phase (~2x matmul slowdown, worth
~80us): hence the software-pipelined PV (consumes the previous kt-pair's
exp), the interleaved final-projection matmuls, and the periodic bf16
keep-warm filler matmuls. Input DMAs are split across both HWDGE rings
and chunked per d-tile so the QKV matmuls start after the first ~1MB.
"""

import sys

sys.path.insert(0, "/opt/trn_rl_repo")

import numpy as np
from contextlib import ExitStack

import concourse.bass as bass
import concourse.tile as tile
from concourse import bacc, mybir
from concourse.bass_utils import run_bass_kernel_spmd


def _install_ntff_hook():
    """Provide antenv.axon_hooks (absent from the image) so
    run_bass_kernel_spmd(trace=True) can capture NTFF profiles under axon."""
    import types

    if "antenv.axon_hooks" in sys.modules:
        return
    mod = types.ModuleType("antenv.axon_hooks")
    mod._HOOK = None
    mod.set_axon_ntff_profile_hook = lambda h: setattr(mod, "_HOOK", h)
    mod.get_axon_ntff_profile_hook = lambda: mod._HOOK
    try:
        from trn_agent_boot.trn_boot import _ntff_profile_via_ctypes

        mod._HOOK = _ntff_profile_via_ctypes("/opt/axon/libaxon_pjrt.so")
    except Exception:
        pass
    sys.modules["antenv.axon_hooks"] = mod
    try:
        import antenv

        antenv.axon_hooks = mod
    except Exception:
        pass


_install_ntff_hook()

f32 = mybir.dt.float32
f32r = mybir.dt.float32r
Exp = mybir.ActivationFunctionType.Exp

# Problem shapes (hardcoded per contract)
B, N, D = 4, 2048, 512
HEADS, DH = 8, 64
INNER = HEADS * DH  # 512
SCALE = DH**-0.5
NCORES = 8
HG = 2  # head groups (tensor-parallel degree)
HPC = HEADS // HG  # 4 heads per core
F = HPC * DH  # 256 inner dims per core
NT = N // 128  # 16 n(token)-tiles
DT = D // 128  # 4 d-tiles
QC = 4  # q chunks of 512
KTP = NT // 2  # 8 kt-pairs

_NC_CACHE = {}


def _build():
    """Build + compile the per-core Bass program (same program on all cores)."""
    nc = bacc.Bacc("TRN2", target_bir_lowering=False, debug=False, num_devices=NCORES)

    xT_d = nc.dram_tensor("xT", [D, N], f32, kind="ExternalInput")
    wq_d = nc.dram_tensor("wq", [D, F], f32, kind="ExternalInput")
    wk_d = nc.dram_tensor("wk", [D, F], f32, kind="ExternalInput")
    wv_d = nc.dram_tensor("wv", [D, F], f32, kind="ExternalInput")  # gate-scaled
    wo_d = nc.dram_tensor("wo", [F, D], f32, kind="ExternalInput")
    ones_d = nc.dram_tensor("ones", [128, 64], f32, kind="ExternalInput")
    part_d = nc.dram_tensor("part", [N, D], f32, kind="ExternalOutput")

    with tile.TileContext(nc) as tc:
        with ExitStack() as ctx:
            persist = ctx.enter_context(tc.tile_pool(name="persist", bufs=1))

            # qT/kT: [f, n] layout, one tile per head-pair (f-tile).
            qt_sb = [
                persist.tile([128, N], f32r, tag=f"qt{i}", name=f"qt{i}")
                for i in range(2)
            ]
            kt_sb = [
                persist.tile([128, N], f32r, tag=f"kt{i}", name=f"kt{i}")
                for i in range(2)
            ]
            # V_ext natural layout: [ktok, nt, head, 64+1]
            v_sb = persist.tile([128, NT, HPC, 65], f32r, tag="v", name="v")
            # gated+normalized attention output O^T: [f, n], per head-pair
            og_sb = [
                persist.tile([128, N], f32r, tag=f"og{i}", name=f"og{i}")
                for i in range(2)
            ]
            wo_sb = persist.tile([128, 2, D], f32r, tag="wo", name="wo")
            wo_bf = persist.tile([128, 512], mybir.dt.bfloat16, tag="wobf", name="wobf")
            ones_sb = persist.tile([1, 64], f32r, tag="ones1", name="ones1")

            ones64_sb = persist.tile([128, 64], f32r, tag="ones64", name="ones64")
            warm_sb = persist.tile([1, 64], f32, tag="warm", name="warm")

            # ---------------- Phase 1: QKV projections ----------------
            # ph1 (xT + weight tiles) stays open through attention: the V
            # projection matmuls are interleaved into the attention stream.
            ph1 = ctx.enter_context(tc.tile_pool(name="ph1", bufs=1))
            with tc.tile_pool(name="ps1", bufs=8, space="PSUM") as ps1:
                # chunk the xT DMA per d-tile AND use one tile per chunk so
                # dependency tracking lets the first matmuls start after ~1MB
                # instead of waiting for the whole 4MB
                xt_sb = [
                    ph1.tile([128, N], f32r, tag=f"xt{dt}", name=f"xt{dt}")
                    for dt in range(DT)
                ]
                w_sb = {}
                for wname, w_d in (("wq", wq_d), ("wk", wk_d), ("wv", wv_d)):
                    w_sb[wname] = [
                        ph1.tile([128, F], f32r, tag=f"{wname}{dt}", name=f"{wname}{dt}")
                        for dt in range(DT)
                    ]
                xt_r = xT_d[:].rearrange("(dt p) n -> p dt n", p=128).bitcast(f32r)
                for dt in range(DT):
                    # weights ride the second HWDGE ring (qActDynamicHW) so
                    # they flow in parallel with the xT chunks on qSPDynamicHW
                    for wname, w_d in (("wq", wq_d), ("wk", wk_d), ("wv", wv_d)):
                        nc.scalar.dma_start(
                            w_sb[wname][dt][:],
                            w_d[:]
                            .rearrange("(dt p) f -> p dt f", p=128)
                            .bitcast(f32r)[:, dt],
                        )
                    nc.sync.dma_start(xt_sb[dt][:], xt_r[:, dt])
                    if dt == 0:
                        # small constants after the first xT chunk; wo on the
                        # weight (qAct) ring; warmup exp loads the ACT table
                        # (~2.7us) well before the first real exp
                        nc.sync.dma_start(ones_sb[:], ones_d[0:1, :].bitcast(f32r))
                        nc.sync.dma_start(ones64_sb[:], ones_d[:, :].bitcast(f32r))
                        nc.scalar.dma_start(
                            wo_sb[:],
                            wo_d[:]
                            .rearrange("(ft p) m -> p ft m", p=128)
                            .bitcast(f32r),
                        )
                        nc.scalar.activation(
                            warm_sb[:], ones_sb[:].bitcast(f32), Exp, scale=0.0
                        )
                        nc.vector.tensor_copy(wo_bf[:], wo_sb[:, 0, :].bitcast(f32))
                        nc.vector.tensor_copy(
                            v_sb[:].rearrange("p a b c -> p (a b) c")[:, :, 64],
                            ones64_sb[:],
                        )

                # qT/kT: [f, n] = W^T x^T ; lhsT = W[dtile, ftile], rhs =
                # xT[dtile, nchunk]. dt-outer over 8 live psum accumulators:
                # PE starts on the first xT chunk.
                def qk_proj(wname, dst):
                    tiles = [
                        ps1.tile([128, 512], f32, tag="mm", name=f"mm_ps{i}")
                        for i in range(8)
                    ]
                    for dt in range(DT):
                        for ft in range(2):
                            for qc in range(QC):
                                nc.tensor.matmul(
                                    tiles[ft * QC + qc][:],
                                    w_sb[wname][dt][:, ft * 128 : (ft + 1) * 128],
                                    xt_sb[dt][:, qc * 512 : (qc + 1) * 512],
                                    start=(dt == 0),
                                    stop=(dt == DT - 1),
                                )
                    for ft in range(2):
                        for qc in range(QC):
                            nc.vector.tensor_copy(
                                dst[ft][:, qc * 512 : (qc + 1) * 512],
                                tiles[ft * QC + qc][:],
                            )

                def v_proj_ps1(nt):
                    ps = ps1.tile([128, 512], f32, tag="mm", name="v0_ps")
                    for dt in range(DT):
                        nc.tensor.matmul(
                            ps[:, 0:F],
                            xt_sb[dt][:, nt * 128 : (nt + 1) * 128],
                            w_sb["wv"][dt][:],
                            start=(dt == 0),
                            stop=(dt == DT - 1),
                        )
                    nc.vector.tensor_copy(
                        v_sb[:, nt, :, 0:64],
                        ps[:, 0:F].rearrange("p (h e) -> p h e", e=64),
                    )

                qk_proj("wq", qt_sb)
                qk_proj("wk", kt_sb)
                for nt in range(NT):
                    v_proj_ps1(nt)


            # ---------------- Phase 2: attention ----------------
            # psS bufs=3 (6 banks) lets PE run ST matmuls ~2 kt-pairs ahead
            # of the exp on ACT, keeping the tensor engine dense enough to
            # hold the HAM clock at 2.4 GHz. psO bufs=2 (2 banks) pipelines
            # the per-(head, qchunk) accumulator across iterations.
            with (
                tc.tile_pool(name="ptp", bufs=6) as ptp,
                tc.tile_pool(name="normp", bufs=2) as normp,
                tc.tile_pool(name="psS", bufs=2, space="PSUM") as psS,
                tc.tile_pool(name="psO", bufs=2, space="PSUM") as psO,
                tc.tile_pool(name="psD", bufs=1, space="PSUM") as psD,
                tc.tile_pool(name="outp", bufs=4) as outp,
            ):

                def final_nt(nt):
                    ps = psD.tile([128, 512], f32, tag="F", name="f_ps")
                    for fhp in range(2):
                        nc.tensor.matmul(
                            ps[:],
                            og_sb[fhp][:, nt * 128 : (nt + 1) * 128],
                            wo_sb[:, fhp, :],
                            start=(fhp == 0),
                            stop=(fhp == 1),
                        )
                    ob = outp.tile([128, 512], f32, tag="ob", name="ob")
                    nc.vector.tensor_copy(ob[:], ps[:])
                    nc.sync.dma_start(part_d[nt * 128 : (nt + 1) * 128, :], ob[:])

                def pv_pair(o_ps, pt, hp, h01, ktp):
                    for j in range(2):
                        kt = 2 * ktp + j
                        nc.tensor.matmul(
                            o_ps[:],
                            v_sb[:, kt, hp * 2 + h01, :],
                            pt[:, j * 512 : (j + 1) * 512],
                            start=(kt == 0),
                            stop=(kt == NT - 1),
                        )

                def normalize(blk, o_ps):
                    # og = O[0:64] * (1 / sums), sums broadcast down 64
                    # partitions via a K=1 matmul with a ones lhsT
                    hp, qc, h01 = blk
                    off = h01 * 64
                    srow = normp.tile([1, 512], f32r, tag="srow", name="srow")
                    nc.vector.tensor_copy(srow[:], o_ps[64:65, :])
                    rs_ps = psD.tile([64, 512], f32, tag="D", name="rs_ps")
                    nc.tensor.matmul(
                        rs_ps[:], ones_sb[:], srow[:], start=True, stop=True
                    )
                    rinv = normp.tile([64, 512], f32, tag="rinv", name="rinv")
                    nc.vector.reciprocal_approx_fast(rinv[:], rs_ps[:])
                    nc.vector.tensor_tensor(
                        og_sb[hp][off : off + 64, qc * 512 : (qc + 1) * 512],
                        o_ps[0:64, :],
                        rinv[:],
                        mybir.AluOpType.mult,
                    )
                    if hp == 1:
                        # queue this q-chunk's final-projection n-tiles once
                        # both head-pairs' og columns exist; drained one per
                        # iteration to keep the PE stream smooth
                        if h01 == 1:
                            pending_finals.extend(range(qc * 4, qc * 4 + 4))

                # One flat software pipeline over all (head-pair, q-chunk,
                # head) blocks and kt-pairs: the PV matmuls consume the
                # PREVIOUS iteration's exp output (even across block
                # boundaries), so the tensor engine never waits on the
                # activation engine; keep-warm filler matmuls top PE pace up
                # to just above the exp pace so the HAM clock gate stays at
                # 2.4 GHz for the whole phase.
                pending_finals = []
                blocks = [
                    (hp, qc, h01)
                    for hp in range(2)
                    for qc in range(QC)
                    for h01 in range(2)
                ]
                prev = None  # (pt, o_ps, blk, ktp)
                o_cur = None
                it_count = 0
                for blk in blocks:
                    hp, qc, h01 = blk
                    off = h01 * 64
                    o_cur = psO.tile([65, 512], f32, tag="O", name="o_ps")
                    for ktp in range(KTP):
                        s_ps = psS.tile([128, 1024], f32, tag="S", name="s_ps")
                        for j in range(2):
                            kt = 2 * ktp + j
                            nc.tensor.matmul(
                                s_ps[:, j * 512 : (j + 1) * 512],
                                kt_sb[hp][off : off + 64, kt * 128 : (kt + 1) * 128],
                                qt_sb[hp][off : off + 64, qc * 512 : (qc + 1) * 512],
                                start=True,
                                stop=True,
                            )
                        n_dummy = 1 if it_count % 3 == 0 else 0
                        it_count += 1
                        if pending_finals:
                            final_nt(pending_finals.pop(0))
                            n_dummy = 0
                        d_ps = psD.tile([128, 512], f32, tag="D", name="d_ps")
                        for _ in range(n_dummy):
                            nc.tensor.matmul(
                                d_ps[:],
                                wo_bf[:, 0:128],
                                wo_bf[:],
                                start=True,
                                stop=True,
                            )
                        pt = ptp.tile([128, 1024], f32r, tag="PT", name="pt")
                        nc.scalar.activation(pt[:], s_ps[:], Exp, scale=SCALE)
                        if prev is not None:
                            p_pt, p_o, p_blk, p_ktp = prev
                            pv_pair(p_o, p_pt, p_blk[0], p_blk[2], p_ktp)
                            if p_ktp == KTP - 1:
                                normalize(p_blk, p_o)
                        prev = (pt, o_cur, blk, ktp)
                p_pt, p_o, p_blk, p_ktp = prev
                pv_pair(p_o, p_pt, p_blk[0], p_blk[2], p_ktp)
                normalize(p_blk, p_o)
                for nt in pending_finals:
                    final_nt(nt)


    nc.compile()
    return nc


def _get_nc():
    if "nc" not in _NC_CACHE:
        _NC_CACHE["nc"] = _build()
    return _NC_CACHE["nc"]


def _prepare_in_maps(x, domain_label, W_qkv, W_d1, b_d1, W_d2, b_d2, W_out, b_out):
    x = np.asarray(x, np.float32)
    domain_label = np.asarray(domain_label, np.float32)
    W_qkv = np.asarray(W_qkv, np.float32)
    W_d1 = np.asarray(W_d1, np.float32)
    b_d1 = np.asarray(b_d1, np.float32)
    W_d2 = np.asarray(W_d2, np.float32)
    b_d2 = np.asarray(b_d2, np.float32)
    W_out = np.asarray(W_out, np.float32)

    # host: domain gate MLP + softmax over heads (tiny)
    d1 = np.maximum(domain_label @ W_d1 + b_d1, 0.0)
    d = d1 @ W_d2 + b_d2  # [B, INNER]
    d = d.reshape(B, HEADS, DH)
    e = np.exp(d - d.max(axis=1, keepdims=True))
    gate = (e / e.sum(axis=1, keepdims=True)).reshape(B, INNER).astype(np.float32)

    ones = np.ones((128, 64), np.float32)
    in_maps = []
    for c in range(NCORES):
        b, g = c // HG, c % HG
        sl = slice(g * F, (g + 1) * F)
        in_maps.append(
            {
                "xT": np.ascontiguousarray(x[b].T),
                "wq": np.ascontiguousarray(W_qkv[:, sl]),
                "wk": np.ascontiguousarray(W_qkv[:, INNER:][:, sl]),
                "wv": np.ascontiguousarray(
                    W_qkv[:, 2 * INNER :][:, sl] * gate[b, sl][None, :]
                ),
                "wo": np.ascontiguousarray(W_out[sl, :]),
                "ones": ones,
            }
        )
    return in_maps


def _run(in_maps, trace=False, tmpdir=None):
    nc = _get_nc()
    return run_bass_kernel_spmd(
        nc, in_maps, list(range(NCORES)), trace=trace, tmpdir=tmpdir
    )


def _assemble(results, b_out):
    b_out = np.asarray(b_out, np.float32)
    out = np.empty((B, N, D), np.float32)
    for b in range(B):
        out[b] = results[HG * b]["part"] + results[HG * b + 1]["part"] + b_out
    return out


def kernel(x, domain_label, W_qkv, W_d1, b_d1, W_d2, b_d2, W_out, b_out):
    in_maps = _prepare_in_maps(
        x, domain_label, W_qkv, W_d1, b_d1, W_d2, b_d2, W_out, b_out
    )
    res = _run(in_maps, trace=False)
    return _assemble(res.results, b_out)



# revision 3
# speedup vs baseline: 1.1536x; 1.1536x over previous
"""Trainium2 Bass kernel for nn_AttentionSup (dense transformer attention block).

Computation (see reference):
  qkv = x @ W_qkv; per-head attention softmax(q k^T / sqrt(d)) v;
  domain-gate (tiny MLP + softmax over heads) multiplies the attention
  output per (batch, head, dim); out = gated @ W_out + b_out.

Sharding over 8 NeuronCores: (batch b in 0..3) x (head-group g in 0..1),
4 heads per core - data-parallel over batch, tensor-parallel over heads.
Each core computes a partial output [2048, 512] for its batch from its 4
heads; the host sums the two head-group partials per batch and adds b_out.

Key performance structure (v2, ACT-roofline design):
  - The exp stream on ScalarE is the hard floor: 16.8M score elements per
    core = 128 x [128,1024] ACTIVATE tiles ~ 1.15us each ~ 147us. The whole
    kernel is scheduled so ACT runs back-to-back exps and everything else
    (PE matmuls, DVE copies, DMA) hides underneath.
  - All on-chip data is bf16 (host pre-casts inputs): halves DMA bytes and
    enables FWL weight loads + LDWEIGHTS prefetch on the PE (fp32r
    self-loading matmuls serialize their ~200ns weight load).
  - The two heads of a head-pair run their K=64 S matmuls CONCURRENTLY in
    PE row-tiles (0,0)/(64,0) (tile_position auto-derived from the
    partition offsets of the kt/qt slices).
  - xT is DMA'd in token chunks and QKV projections are chunk-pipelined so
    the first exp lands ~10us in instead of ~54us; remaining projections
    are drained as "extras" under the early exp stream.
  - softmax normalization via the V|1 ones-column trick (PV matmul also
    yields row sums), reciprocal broadcast multiply; exp skips
    max-subtraction (scores ~N(0,1)).
  - Filler matmuls keep the PE HAM clock gate at 2.4 GHz (they write to
    unused partitions 96:128 of the PSUM O-accumulator banks).
"""

import sys

sys.path.insert(0, "/opt/trn_rl_repo")

import numpy as np
import ml_dtypes
from contextlib import ExitStack

import concourse.bass as bass
import concourse.tile as tile
from concourse import bacc, mybir
from concourse.bass_utils import run_bass_kernel_spmd


def _install_ntff_hook():
    """Provide antenv.axon_hooks (absent from the image) so
    run_bass_kernel_spmd(trace=True) can capture NTFF profiles under axon."""
    import types

    if "antenv.axon_hooks" in sys.modules:
        return
    mod = types.ModuleType("antenv.axon_hooks")
    mod._HOOK = None
    mod.set_axon_ntff_profile_hook = lambda h: setattr(mod, "_HOOK", h)
    mod.get_axon_ntff_profile_hook = lambda: mod._HOOK
    try:
        from trn_agent_boot.trn_boot import _ntff_profile_via_ctypes

        mod._HOOK = _ntff_profile_via_ctypes("/opt/axon/libaxon_pjrt.so")
    except Exception:
        pass
    sys.modules["antenv.axon_hooks"] = mod
    try:
        import antenv

        antenv.axon_hooks = mod
    except Exception:
        pass


_install_ntff_hook()

f32 = mybir.dt.float32
f32r = mybir.dt.float32r
bf16 = mybir.dt.bfloat16
Exp = mybir.ActivationFunctionType.Exp
BF = ml_dtypes.bfloat16

# Problem shapes (hardcoded per contract)
B, N, D = 4, 2048, 512
HEADS, DH = 8, 64
INNER = HEADS * DH  # 512
SCALE = DH**-0.5
NCORES = 8
HG = 2  # head groups (tensor-parallel degree)
HPC = HEADS // HG  # 4 heads per core
F = HPC * DH  # 256 inner dims per core
NT = N // 128  # 16 key tiles
DT = D // 128  # 4 d-tiles (contraction)
CH = 4  # token chunks (both q-chunks and kt/v chunks)
CW = N // CH  # 512 chunk width
QC = CH
KTP = NT // 2  # 8 key-tile-pairs per (head-pair, q-chunk) group

N_WARM_FILL = 10  # HAM warmup matmuls at t0
FILLER_N = 3  # steady-state keep-warm matmuls per iteration

_NC_CACHE = {}


def _build():
    """Build + compile the per-core Bass program (same program on all cores)."""
    nc = bacc.Bacc("TRN2", target_bir_lowering=False, debug=False, num_devices=NCORES)

    xT_d = nc.dram_tensor("xT", [D, N], bf16, kind="ExternalInput")
    # wall = [wq | wk | wv(gate-scaled)] packed per d-row: [D, 3F]
    wall_d = nc.dram_tensor("wall", [D, 3 * F], bf16, kind="ExternalInput")
    wo_d = nc.dram_tensor("wo", [F, D], bf16, kind="ExternalInput")
    ones_d = nc.dram_tensor("ones", [128, 64], bf16, kind="ExternalInput")
    ones32_d = nc.dram_tensor("ones32", [1, 64], f32, kind="ExternalInput")
    part_d = nc.dram_tensor("part", [N, D], f32, kind="ExternalOutput")

    with tile.TileContext(nc) as tc:
        with ExitStack() as ctx:
            persist = ctx.enter_context(tc.tile_pool(name="persist", bufs=1))

            # projections, per (head-pair, chunk): [f 128, tok 512]
            qt = [
                [
                    persist.tile([128, CW], bf16, tag=f"qt{hp}_{c}", name=f"qt{hp}_{c}")
                    for c in range(CH)
                ]
                for hp in range(HG)
            ]
            kt = [
                [
                    persist.tile([128, CW], bf16, tag=f"kt{hp}_{c}", name=f"kt{hp}_{c}")
                    for c in range(CH)
                ]
                for hp in range(HG)
            ]
            # V_ext per key tile: [ktok 128, head 4, 64+1]
            vt = [
                persist.tile([128, HPC, 65], bf16, tag=f"v{t}", name=f"v{t}")
                for t in range(NT)
            ]
            # gated+normalized attention output O^T per (head-pair, q-chunk)
            og = [
                [
                    persist.tile([128, CW], bf16, tag=f"og{hp}_{q}", name=f"og{hp}_{q}")
                    for q in range(QC)
                ]
                for hp in range(HG)
            ]
            wo_sb = persist.tile([128, 2, D], bf16, tag="wo", name="wo_sb")
            w_sb = [
                persist.tile([128, 3 * F], bf16, tag=f"w{dt}", name=f"w{dt}")
                for dt in range(DT)
            ]
            xt = [
                [
                    persist.tile([128, CW], bf16, tag=f"xt{c}_{dt}", name=f"xt{c}_{dt}")
                    for dt in range(DT)
                ]
                for c in range(CH)
            ]
            ones1 = persist.tile([1, 64], f32r, tag="ones1", name="ones1")
            ones64 = persist.tile([128, 64], bf16, tag="ones64", name="ones64")
            fil = persist.tile([128, 512], bf16, tag="fil", name="fil")
            warm = persist.tile([1, 64], f32, tag="warm", name="warm")

            ptp = ctx.enter_context(tc.tile_pool(name="ptp", bufs=6))
            normp = ctx.enter_context(tc.tile_pool(name="normp", bufs=2))
            outp = ctx.enter_context(tc.tile_pool(name="outp", bufs=4))
            # PSUM budget (8 banks): psS 2x[128,1024] = 4, psO 2x[128,512] = 2,
            # psD 2x[128,512] = 2 (shared by projections / rs / finals).
            psS = ctx.enter_context(tc.tile_pool(name="psS", bufs=2, space="PSUM"))
            psO = ctx.enter_context(tc.tile_pool(name="psO", bufs=2, space="PSUM"))
            psD = ctx.enter_context(tc.tile_pool(name="psD", bufs=2, space="PSUM"))

            # ---------------- t0: warmup + DMA kickoff ----------------
            nc.vector.memset(fil[:], 1.0)
            # loads the ACT exp table (~2.7us) under the input DMA
            nc.scalar.activation(warm[:], fil[0:1, 0:64], Exp, scale=0.0)

            wall_r = wall_d[:].rearrange("(dt p) f -> p dt f", p=128)
            for dt in range(DT):
                nc.scalar.dma_start(w_sb[dt][:], wall_r[:, dt])
            nc.scalar.dma_start(
                wo_sb[:], wo_d[:].rearrange("(ft p) m -> p ft m", p=128)
            )
            nc.scalar.dma_start(ones64[:], ones_d[:])
            nc.scalar.dma_start(ones1[:], ones32_d[0:1, :].bitcast(f32r))
            xt_r = xT_d[:].rearrange("(dt p) n -> p dt n", p=128)
            for c in range(CH):
                for dt in range(DT):
                    nc.sync.dma_start(
                        xt[c][dt][:], xt_r[:, dt, c * CW : (c + 1) * CW]
                    )

            # HAM warmup: ~4us of dependency-free matmuls so the first real
            # projections run at 2.4 GHz
            for i in range(N_WARM_FILL):
                w_ps = psD.tile([128, 512], f32, tag="D", name=f"wf{i}")
                nc.tensor.matmul(
                    w_ps[:], fil[:, 0:128], fil[:], start=True, stop=True
                )

            # ---------------- projection helpers ----------------
            def proj_qk(which, hp, c):
                # [f 128, tok 512] = W^T x^T; lhsT = W[dt, f-slice], rhs = xT[dt, chunk]
                off = (0 if which == "q" else F) + hp * 128
                ps = psD.tile([128, 512], f32, tag="D", name=f"p{which}{hp}{c}")
                for dt in range(DT):
                    nc.tensor.matmul(
                        ps[:],
                        w_sb[dt][:, off : off + 128],
                        xt[c][dt][:],
                        start=(dt == 0),
                        stop=(dt == DT - 1),
                    )
                dst = (qt if which == "q" else kt)[hp][c]
                nc.vector.tensor_copy(dst[:], ps[:])

            def proj_v(t):
                # [tok 128, f 256]; lhsT = xT[dt, tok-tile], rhs = Wv[dt]
                c, ti = t // 4, t % 4
                ps = psD.tile([128, 512], f32, tag="D", name=f"pv{t}")
                for dt in range(DT):
                    nc.tensor.matmul(
                        ps[:, 0:F],
                        xt[c][dt][:, ti * 128 : (ti + 1) * 128],
                        w_sb[dt][:, 2 * F : 3 * F],
                        start=(dt == 0),
                        stop=(dt == DT - 1),
                    )
                nc.vector.tensor_copy(
                    vt[t][:, :, 0:64],
                    ps[:, 0:F].rearrange("p (h e) -> p h e", e=64),
                )
                nc.vector.tensor_copy(vt[t][:, :, 64], ones64[:, 0:HPC])

            def final_nt(nt):
                fp = psD.tile([128, 512], f32, tag="D", name=f"f{nt}")
                qcn, ti = nt // 4, nt % 4
                for hp2 in range(HG):
                    nc.tensor.matmul(
                        fp[:],
                        og[hp2][qcn][:, ti * 128 : (ti + 1) * 128],
                        wo_sb[:, hp2, :],
                        start=(hp2 == 0),
                        stop=(hp2 == HG - 1),
                    )
                ob = outp.tile([128, 512], f32, tag="ob", name=f"ob{nt}")
                nc.vector.tensor_copy(ob[:], fp[:])
                nc.sync.dma_start(part_d[nt * 128 : (nt + 1) * 128, :], ob[:])

            def filler(o_pair, i):
                # keep-warm matmul into unused partitions 96:128 of the live
                # O-accumulator bank (never read; disjoint from PV's rows 0:65)
                nc.tensor.matmul(
                    o_pair[i % 2][96:128, :],
                    fil[:, 0:32],
                    fil[:],
                    start=True,
                    stop=True,
                    skip_group_check=True,
                    tile_position=(0, 96),
                )

            def normalize(grp, o_pair):
                qcn, hp = grp
                for h01 in range(2):
                    srow = normp.tile([1, 512], f32r, tag="sr", name=f"sr{h01}")
                    nc.vector.tensor_copy(srow[:], o_pair[h01][64:65, :])
                    rs = psD.tile([64, 512], f32, tag="D", name=f"rs{h01}")
                    nc.tensor.matmul(rs[:], ones1[:], srow[:], start=True, stop=True)
                    rinv = normp.tile([64, 512], f32, tag="ri", name=f"ri{h01}")
                    nc.vector.reciprocal_approx_fast(rinv[:], rs[:])
                    nc.vector.tensor_tensor(
                        og[hp][qcn][h01 * 64 : (h01 + 1) * 64, :],
                        o_pair[h01][0:64, :],
                        rinv[:],
                        mybir.AluOpType.mult,
                    )

            def pv_prev(prev):
                pts, o_pair, (qcn, hp), ktp = prev
                for j in range(2):
                    kti = 2 * ktp + j
                    for h01 in range(2):
                        nc.tensor.matmul(
                            o_pair[h01][0:65, :],
                            vt[kti][:, hp * 2 + h01, :],
                            pts[h01][:, j * 512 : (j + 1) * 512],
                            start=(kti == 0),
                            stop=(kti == NT - 1),
                        )

            # ---------------- pre-loop projections ----------------
            proj_qk("k", 0, 0)
            proj_qk("q", 0, 0)
            proj_v(0)
            proj_v(1)

            # remaining projection work, scheduled by first-use deadline
            extras = {
                0: [lambda: proj_v(2), lambda: proj_v(3)],
                1: [lambda: proj_v(4), lambda: proj_v(5), lambda: proj_qk("k", 0, 1)],
                2: [lambda: proj_v(6), lambda: proj_v(7)],
                3: [lambda: proj_v(8), lambda: proj_v(9), lambda: proj_qk("k", 0, 2)],
                4: [lambda: proj_v(10), lambda: proj_v(11)],
                5: [
                    lambda: proj_v(12),
                    lambda: proj_v(13),
                    lambda: proj_qk("k", 0, 3),
                ],
                6: [
                    lambda: proj_v(14),
                    lambda: proj_v(15),
                    lambda: proj_qk("k", 1, 0),
                ],
                7: [lambda: proj_qk("q", 1, 0)],
                9: [lambda: proj_qk("k", 1, 1)],
                11: [lambda: proj_qk("k", 1, 2)],
                13: [lambda: proj_qk("k", 1, 3)],
                14: [lambda: proj_qk("q", 0, 1)],
                22: [lambda: proj_qk("q", 1, 1)],
                30: [lambda: proj_qk("q", 0, 2)],
                38: [lambda: proj_qk("q", 1, 2)],
                46: [lambda: proj_qk("q", 0, 3)],
                54: [lambda: proj_qk("q", 1, 3)],
            }

            # ---------------- attention: flat software-pipelined loop ----
            groups = [(qcn, hp) for qcn in range(QC) for hp in range(HG)]
            pending_finals = []
            prev = None  # (pts, o_pair, grp, ktp)
            it = 0
            for gi, grp in enumerate(groups):
                qcn, hp = grp
                o_pair = [
                    psO.tile([128, 512], f32, tag="O", name=f"o{gi}_{h}")
                    for h in range(2)
                ]
                for ktp in range(KTP):
                    # S^T tiles for both heads of the pair, row-tiled so the
                    # two heads' K=64 matmuls run concurrently in the PE
                    ps_pair = [
                        psS.tile([128, 1024], f32, tag="S", name=f"s{it}_{h}")
                        for h in range(2)
                    ]
                    for j in range(2):
                        kti = 2 * ktp + j
                        c, ti = kti // 4, kti % 4
                        for h01 in range(2):
                            nc.tensor.matmul(
                                ps_pair[h01][:, j * 512 : (j + 1) * 512],
                                kt[hp][c][
                                    h01 * 64 : (h01 + 1) * 64,
                                    ti * 128 : (ti + 1) * 128,
                                ],
                                qt[hp][qcn][h01 * 64 : (h01 + 1) * 64, :],
                                start=True,
                                stop=True,
                            )

                    # exp on ACT (the pacer) - issue right after S
                    pts = []
                    for h01 in range(2):
                        pt = ptp.tile(
                            [128, 1024], bf16, tag="PT", name=f"pt{it}_{h01}"
                        )
                        nc.scalar.activation(
                            pt[:], ps_pair[h01][:], Exp, scale=SCALE
                        )
                        pts.append(pt)

                    # PE side work for this iteration slot
                    ex = extras.pop(it, [])
                    for fn in ex:
                        fn()
                    nfil = 0
                    if not ex:
                        nfil = FILLER_N
                        if pending_finals:
                            final_nt(pending_finals.pop(0))
                            nfil -= 2
                    for i in range(max(nfil, 0)):
                        filler(o_pair, i)

                    # PV of the previous iteration's exp output
                    if prev is not None:
                        pv_prev(prev)
                        if prev[3] == KTP - 1:
                            normalize(prev[2], prev[1])
                            if prev[2][1] == HG - 1:
                                pending_finals.extend(
                                    range(prev[2][0] * 4, prev[2][0] * 4 + 4)
                                )
                    prev = (pts, o_pair, grp, ktp)
                    it += 1

            # tail
            pv_prev(prev)
            normalize(prev[2], prev[1])
            pending_finals.extend(range(prev[2][0] * 4, prev[2][0] * 4 + 4))
            for nt in pending_finals:
                final_nt(nt)

    nc.compile()
    return nc


def _get_nc():
    if "nc" not in _NC_CACHE:
        _NC_CACHE["nc"] = _build()
    return _NC_CACHE["nc"]


def _prepare_in_maps(x, domain_label, W_qkv, W_d1, b_d1, W_d2, b_d2, W_out, b_out):
    x = np.asarray(x, np.float32)
    domain_label = np.asarray(domain_label, np.float32)
    W_qkv = np.asarray(W_qkv, np.float32)
    W_d1 = np.asarray(W_d1, np.float32)
    b_d1 = np.asarray(b_d1, np.float32)
    W_d2 = np.asarray(W_d2, np.float32)
    b_d2 = np.asarray(b_d2, np.float32)
    W_out = np.asarray(W_out, np.float32)

    # host: domain gate MLP + softmax over heads (tiny)
    d1 = np.maximum(domain_label @ W_d1 + b_d1, 0.0)
    d = d1 @ W_d2 + b_d2  # [B, INNER]
    d = d.reshape(B, HEADS, DH)
    e = np.exp(d - d.max(axis=1, keepdims=True))
    gate = (e / e.sum(axis=1, keepdims=True)).reshape(B, INNER).astype(np.float32)

    ones = np.ones((128, 64), BF)
    ones32 = np.ones((1, 64), np.float32)
    in_maps = []
    for c in range(NCORES):
        b, g = c // HG, c % HG
        sl = slice(g * F, (g + 1) * F)
        wq = W_qkv[:, :INNER][:, sl]
        wk = W_qkv[:, INNER : 2 * INNER][:, sl]
        wv = W_qkv[:, 2 * INNER :][:, sl] * gate[b, sl][None, :]
        wall = np.ascontiguousarray(
            np.concatenate([wq, wk, wv], axis=1).astype(BF)
        )
        in_maps.append(
            {
                "xT": np.ascontiguousarray(x[b].T.astype(BF)),
                "wall": wall,
                "wo": np.ascontiguousarray(W_out[sl, :].astype(BF)),
                "ones": ones,
                "ones32": ones32,
            }
        )
    return in_maps


def _run(in_maps, trace=False, tmpdir=None):
    nc = _get_nc()
    return run_bass_kernel_spmd(
        nc, in_maps, list(range(NCORES)), trace=trace, tmpdir=tmpdir
    )


def _assemble(results, b_out):
    b_out = np.asarray(b_out, np.float32)
    out = np.empty((B, N, D), np.float32)
    for b in range(B):
        out[b] = results[HG * b]["part"] + results[HG * b + 1]["part"] + b_out
    return out


def kernel(x, domain_label, W_qkv, W_d1, b_d1, W_d2, b_d2, W_out, b_out):
    in_maps = _prepare_in_maps(
        x, domain_label, W_qkv, W_d1, b_d1, W_d2, b_d2, W_out, b_out
    )
    res = _run(in_maps, trace=False)
    return _assemble(res.results, b_out)


# revision 8
# speedup vs baseline: 1.4936x; 1.2947x over previous
"""Trainium2 Bass kernel for nn_AttentionSup (dense transformer attention block).

Computation (see reference):
  qkv = x @ W_qkv; per-head attention softmax(q k^T / sqrt(d)) v;
  domain-gate (tiny MLP + softmax over heads) multiplies the attention
  output per (batch, head, dim); out = gated @ W_out + b_out.

Sharding over 8 NeuronCores: (batch b in 0..3) x (head-group g in 0..1),
4 heads per core - data-parallel over batch, tensor-parallel over heads.
Each core computes a partial output [2048, 512] for its batch from its 4
heads; the host sums the two head-group partials per batch and adds b_out.

Key performance structure (v2, ACT-roofline design):
  - The exp stream on ScalarE is the hard floor: 16.8M score elements per
    core = 128 x [128,1024] ACTIVATE tiles ~ 1.15us each ~ 147us. The whole
    kernel is scheduled so ACT runs back-to-back exps and everything else
    (PE matmuls, DVE copies, DMA) hides underneath.
  - All on-chip data is bf16 (host pre-casts inputs): halves DMA bytes and
    enables FWL weight loads + LDWEIGHTS prefetch on the PE (fp32r
    self-loading matmuls serialize their ~200ns weight load).
  - The two heads of a head-pair run their K=64 S matmuls CONCURRENTLY in
    PE row-tiles (0,0)/(64,0) (tile_position auto-derived from the
    partition offsets of the kt/qt slices).
  - xT is DMA'd in token chunks and QKV projections are chunk-pipelined so
    the first exp lands ~10us in instead of ~54us; remaining projections
    are drained as "extras" under the early exp stream.
  - softmax normalization via the V|1 ones-column trick (PV matmul also
    yields row sums), reciprocal broadcast multiply; exp skips
    max-subtraction (scores ~N(0,1)).
  - Filler matmuls keep the PE HAM clock gate at 2.4 GHz (they write to
    unused partitions 96:128 of the PSUM O-accumulator banks).
"""

import sys

sys.path.insert(0, "/opt/trn_rl_repo")

import numpy as np
import ml_dtypes
from contextlib import ExitStack

import concourse.bass as bass
import concourse.tile as tile
from concourse import bacc, mybir
from concourse.bass_utils import run_bass_kernel_spmd


def _install_ntff_hook():
    """Provide antenv.axon_hooks (absent from the image) so
    run_bass_kernel_spmd(trace=True) can capture NTFF profiles under axon."""
    import types

    if "antenv.axon_hooks" in sys.modules:
        return
    mod = types.ModuleType("antenv.axon_hooks")
    mod._HOOK = None
    mod.set_axon_ntff_profile_hook = lambda h: setattr(mod, "_HOOK", h)
    mod.get_axon_ntff_profile_hook = lambda: mod._HOOK
    try:
        from trn_agent_boot.trn_boot import _ntff_profile_via_ctypes

        mod._HOOK = _ntff_profile_via_ctypes("/opt/axon/libaxon_pjrt.so")
    except Exception:
        pass
    sys.modules["antenv.axon_hooks"] = mod
    try:
        import antenv

        antenv.axon_hooks = mod
    except Exception:
        pass


_install_ntff_hook()

f32 = mybir.dt.float32
f32r = mybir.dt.float32r
bf16 = mybir.dt.bfloat16
Exp = mybir.ActivationFunctionType.Exp
BF = ml_dtypes.bfloat16

# Problem shapes (hardcoded per contract)
B, N, D = 4, 2048, 512
HEADS, DH = 8, 64
INNER = HEADS * DH  # 512
SCALE = DH**-0.5
NCORES = 8
HG = 2  # head groups (tensor-parallel degree)
HPC = HEADS // HG  # 4 heads per core
F = HPC * DH  # 256 inner dims per core
NT = N // 128  # 16 key tiles
DT = D // 128  # 4 d-tiles (contraction)
CH = 4  # token chunks (both q-chunks and kt/v chunks)
CW = N // CH  # 512 chunk width
QC = CH
KTP = NT // 2  # 8 key-tile-pairs per (head-pair, q-chunk) group

N_WARM_FILL = 10  # HAM warmup matmuls at t0
FILLER_N = 0  # steady-state keep-warm matmuls per iteration

_NC_CACHE = {}


def _build():
    """Build + compile the per-core Bass program (same program on all cores)."""
    nc = bacc.Bacc("TRN2", target_bir_lowering=False, debug=False, num_devices=NCORES)

    xT_d = nc.dram_tensor("xT", [D, N], bf16, kind="ExternalInput")
    # wall = [wq | wk | wv(gate-scaled)] packed per d-row: [D, 3F]
    wall_d = nc.dram_tensor("wall", [D, 3 * F], bf16, kind="ExternalInput")
    wo_d = nc.dram_tensor("wo", [F, D], bf16, kind="ExternalInput")
    ones_d = nc.dram_tensor("ones", [128, 64], bf16, kind="ExternalInput")
    ones32_d = nc.dram_tensor("ones32", [1, 64], f32, kind="ExternalInput")
    part_d = nc.dram_tensor("part", [N, D], f32, kind="ExternalOutput")

    with tile.TileContext(nc) as tc:
        with ExitStack() as ctx:
            persist = ctx.enter_context(tc.tile_pool(name="persist", bufs=1))

            # projections, per (head-pair, chunk): [f 128, tok 512]
            qt = [
                [
                    persist.tile([128, CW], bf16, tag=f"qt{hp}_{c}", name=f"qt{hp}_{c}")
                    for c in range(CH)
                ]
                for hp in range(HG)
            ]
            kt = [
                [
                    persist.tile([128, CW], bf16, tag=f"kt{hp}_{c}", name=f"kt{hp}_{c}")
                    for c in range(CH)
                ]
                for hp in range(HG)
            ]
            # V_ext per key tile: [ktok 128, head 4, 64+1]
            vt = [
                persist.tile([128, HPC, 65], bf16, tag=f"v{t}", name=f"v{t}")
                for t in range(NT)
            ]
            # gated+normalized attention output O^T per (head-pair, q-chunk)
            og = [
                [
                    persist.tile([128, CW], bf16, tag=f"og{hp}_{q}", name=f"og{hp}_{q}")
                    for q in range(QC)
                ]
                for hp in range(HG)
            ]
            wo_sb = persist.tile([128, 2, D], bf16, tag="wo", name="wo_sb")
            w_sb = [
                persist.tile([128, 3 * F], bf16, tag=f"w{dt}", name=f"w{dt}")
                for dt in range(DT)
            ]
            xt = [
                [
                    persist.tile([128, CW], bf16, tag=f"xt{c}_{dt}", name=f"xt{c}_{dt}")
                    for dt in range(DT)
                ]
                for c in range(CH)
            ]
            ones1 = persist.tile([1, 64], f32r, tag="ones1", name="ones1")
            ones64 = persist.tile([128, 64], bf16, tag="ones64", name="ones64")
            fil = persist.tile([128, 512], bf16, tag="fil", name="fil")
            warm = persist.tile([1, 64], f32, tag="warm", name="warm")

            ptp = ctx.enter_context(tc.tile_pool(name="ptp", bufs=6))
            normp = ctx.enter_context(tc.tile_pool(name="normp", bufs=2))
            outp = ctx.enter_context(tc.tile_pool(name="outp", bufs=4))
            # PSUM budget (8 banks): psS 2x[128,1024] = 4, psO 2x[128,512] = 2,
            # psD 2x[128,512] = 2 (shared by projections / rs / finals).
            psS = ctx.enter_context(tc.tile_pool(name="psS", bufs=2, space="PSUM"))
            psO = ctx.enter_context(tc.tile_pool(name="psO", bufs=2, space="PSUM"))
            psD = ctx.enter_context(tc.tile_pool(name="psD", bufs=2, space="PSUM"))

            # ---------------- t0: warmup + DMA kickoff ----------------
            nc.vector.memset(fil[:], 1.0)
            # loads the ACT exp table (~2.7us) under the input DMA
            nc.scalar.activation(warm[:], fil[0:1, 0:64], Exp, scale=0.0)

            wall_r = wall_d[:].rearrange("(dt p) f -> p dt f", p=128)
            for dt in range(DT):
                nc.scalar.dma_start(w_sb[dt][:], wall_r[:, dt])
            nc.scalar.dma_start(
                wo_sb[:], wo_d[:].rearrange("(ft p) m -> p ft m", p=128)
            )
            nc.scalar.dma_start(ones64[:], ones_d[:])
            nc.scalar.dma_start(ones1[:], ones32_d[0:1, :].bitcast(f32r))
            xt_r = xT_d[:].rearrange("(dt p) n -> p dt n", p=128)
            for c in range(CH):
                for dt in range(DT):
                    nc.sync.dma_start(
                        xt[c][dt][:], xt_r[:, dt, c * CW : (c + 1) * CW]
                    )

            # HAM warmup: ~4us of dependency-free matmuls so the first real
            # projections run at 2.4 GHz
            for i in range(N_WARM_FILL):
                w_ps = psD.tile([128, 512], f32, tag="D", name=f"wf{i}")
                nc.tensor.matmul(
                    w_ps[:], fil[:, 0:128], fil[:], start=True, stop=True
                )

            # ---------------- projection helpers ----------------
            def proj_qk(which, hp, c):
                # [f 128, tok 512] = W^T x^T; lhsT = W[dt, f-slice], rhs = xT[dt, chunk]
                off = (0 if which == "q" else F) + hp * 128
                ps = psD.tile([128, 512], f32, tag="D", name=f"p{which}{hp}{c}")
                for dt in range(DT):
                    nc.tensor.matmul(
                        ps[:],
                        w_sb[dt][:, off : off + 128],
                        xt[c][dt][:],
                        start=(dt == 0),
                        stop=(dt == DT - 1),
                    )
                dst = (qt if which == "q" else kt)[hp][c]
                nc.vector.tensor_copy(dst[:], ps[:])

            def proj_v(t):
                # [tok 128, f 256]; lhsT = xT[dt, tok-tile], rhs = Wv[dt]
                c, ti = t // 4, t % 4
                ps = psD.tile([128, 512], f32, tag="D", name=f"pv{t}")
                for dt in range(DT):
                    nc.tensor.matmul(
                        ps[:, 0:F],
                        xt[c][dt][:, ti * 128 : (ti + 1) * 128],
                        w_sb[dt][:, 2 * F : 3 * F],
                        start=(dt == 0),
                        stop=(dt == DT - 1),
                    )
                nc.vector.tensor_copy(
                    vt[t][:, :, 0:64],
                    ps[:, 0:F].rearrange("p (h e) -> p h e", e=64),
                )
                nc.vector.tensor_copy(vt[t][:, :, 64], ones64[:, 0:HPC])

            def final_nt(nt):
                fp = psD.tile([128, 512], f32, tag="D", name=f"f{nt}")
                qcn, ti = nt // 4, nt % 4
                for hp2 in range(HG):
                    nc.tensor.matmul(
                        fp[:],
                        og[hp2][qcn][:, ti * 128 : (ti + 1) * 128],
                        wo_sb[:, hp2, :],
                        start=(hp2 == 0),
                        stop=(hp2 == HG - 1),
                    )
                ob = outp.tile([128, 512], f32, tag="ob", name=f"ob{nt}")
                nc.vector.tensor_copy(ob[:], fp[:])
                nc.sync.dma_start(part_d[nt * 128 : (nt + 1) * 128, :], ob[:])

            def filler(o_pair, i):
                # keep-warm matmul into unused partitions 96:128 of the live
                # O-accumulator bank (never read; disjoint from PV's rows 0:65)
                nc.tensor.matmul(
                    o_pair[i % 2][96:128, :],
                    fil[:, 0:32],
                    fil[:],
                    start=True,
                    stop=True,
                    skip_group_check=True,
                    tile_position=(0, 96),
                )

            def normalize(grp, o_pair, h01):
                qcn, hp = grp
                srow = normp.tile([1, 512], f32r, tag="sr", name=f"sr{h01}")
                nc.vector.tensor_copy(srow[:], o_pair[h01][64:65, :])
                rs = psD.tile([64, 512], f32, tag="D", name=f"rs{h01}")
                nc.tensor.matmul(rs[:], ones1[:], srow[:], start=True, stop=True)
                rinv = normp.tile([64, 512], f32, tag="ri", name=f"ri{h01}")
                nc.vector.reciprocal_approx_fast(rinv[:], rs[:])
                nc.vector.tensor_tensor(
                    og[hp][qcn][h01 * 64 : (h01 + 1) * 64, :],
                    o_pair[h01][0:64, :],
                    rinv[:],
                    mybir.AluOpType.mult,
                )

            def pv_prev(prev):
                pts, o_pair, (qcn, hp), ktp = prev
                for j in range(2):
                    kti = 2 * ktp + j
                    for h01 in range(2):
                        nc.tensor.matmul(
                            o_pair[h01][0:65, :],
                            vt[kti][:, hp * 2 + h01, :],
                            pts[h01][:, j * 512 : (j + 1) * 512],
                            start=(kti == 0),
                            stop=(kti == NT - 1),
                        )

            # ---------------- pre-loop projections ----------------
            proj_qk("k", 0, 0)
            proj_qk("q", 0, 0)
            proj_v(0)
            proj_v(1)

            # remaining projection work, scheduled by first-use deadline
            extras = {
                0: [lambda: proj_v(2), lambda: proj_v(3)],
                1: [lambda: proj_v(4), lambda: proj_v(5), lambda: proj_qk("k", 0, 1)],
                2: [lambda: proj_v(6), lambda: proj_v(7)],
                3: [lambda: proj_v(8), lambda: proj_v(9), lambda: proj_qk("k", 0, 2)],
                4: [lambda: proj_v(10), lambda: proj_v(11)],
                5: [
                    lambda: proj_v(12),
                    lambda: proj_v(13),
                    lambda: proj_qk("k", 0, 3),
                ],
                6: [
                    lambda: proj_v(14),
                    lambda: proj_v(15),
                    lambda: proj_qk("k", 1, 0),
                ],
                7: [lambda: proj_qk("q", 1, 0)],
                9: [lambda: proj_qk("k", 1, 1)],
                11: [lambda: proj_qk("k", 1, 2)],
                13: [lambda: proj_qk("k", 1, 3)],
                14: [lambda: proj_qk("q", 0, 1)],
                22: [lambda: proj_qk("q", 1, 1)],
                30: [lambda: proj_qk("q", 0, 2)],
                38: [lambda: proj_qk("q", 1, 2)],
                46: [lambda: proj_qk("q", 0, 3)],
                54: [lambda: proj_qk("q", 1, 3)],
            }

            # ---------------- attention: flat software-pipelined loop ----
            groups = [(qcn, hp) for qcn in range(QC) for hp in range(HG)]
            pending_finals = []
            pending_norm = None  # (grp, o_pair) awaiting its h1 normalize
            prev = None  # (pts, o_pair, grp, ktp)
            it = 0
            for gi, grp in enumerate(groups):
                qcn, hp = grp
                o_pair = [
                    psO.tile([128, 512], f32, tag="O", name=f"o{gi}_{h}")
                    for h in range(2)
                ]
                for ktp in range(KTP):
                    # S^T tiles for both heads of the pair, row-tiled so the
                    # two heads' K=64 matmuls run concurrently in the PE
                    ps_pair = [
                        psS.tile([128, 1024], f32, tag="S", name=f"s{it}_{h}")
                        for h in range(2)
                    ]
                    for j in range(2):
                        kti = 2 * ktp + j
                        c, ti = kti // 4, kti % 4
                        for h01 in range(2):
                            nc.tensor.matmul(
                                ps_pair[h01][:, j * 512 : (j + 1) * 512],
                                kt[hp][c][
                                    h01 * 64 : (h01 + 1) * 64,
                                    ti * 128 : (ti + 1) * 128,
                                ],
                                qt[hp][qcn][h01 * 64 : (h01 + 1) * 64, :],
                                start=True,
                                stop=True,
                            )

                    # exp on ACT (the pacer) - issue right after S
                    pts = []
                    for h01 in range(2):
                        pt = ptp.tile(
                            [128, 1024], bf16, tag="PT", name=f"pt{it}_{h01}"
                        )
                        nc.scalar.activation(
                            pt[:], ps_pair[h01][:], Exp, scale=SCALE
                        )
                        pts.append(pt)

                    # deferred h1 normalize of the group finished 2 slots ago
                    # (must precede this iteration's PV, which reuses its
                    # psO buffer at the next group boundary)
                    if pending_norm is not None:
                        n_grp, n_opair = pending_norm
                        normalize(n_grp, n_opair, 1)
                        if n_grp[1] == HG - 1:
                            pending_finals.extend(
                                range(n_grp[0] * 4, n_grp[0] * 4 + 4)
                            )
                        pending_norm = None

                    # PV of the previous iteration's exp output
                    if prev is not None:
                        pv_prev(prev)
                        if prev[3] == KTP - 1:
                            normalize(prev[2], prev[1], 0)
                            pending_norm = (prev[2], prev[1])

                    # remaining PE-side work for this slot
                    ex = extras.pop(it, [])
                    for fn in ex:
                        fn()
                    if not ex and pending_finals:
                        final_nt(pending_finals.pop(0))
                    for i in range(FILLER_N):
                        filler(o_pair, i)

                    prev = (pts, o_pair, grp, ktp)
                    it += 1

            # tail
            if pending_norm is not None:
                normalize(pending_norm[0], pending_norm[1], 1)
            pv_prev(prev)
            normalize(prev[2], prev[1], 0)
            normalize(prev[2], prev[1], 1)
            pending_finals.extend(range(prev[2][0] * 4, prev[2][0] * 4 + 4))
            for nt in pending_finals:
                final_nt(nt)

    nc.compile()
    return nc


def _get_nc():
    if "nc" not in _NC_CACHE:
        _NC_CACHE["nc"] = _build()
    return _NC_CACHE["nc"]


def _prepare_in_maps(x, domain_label, W_qkv, W_d1, b_d1, W_d2, b_d2, W_out, b_out):
    x = np.asarray(x, np.float32)
    domain_label = np.asarray(domain_label, np.float32)
    W_qkv = np.asarray(W_qkv, np.float32)
    W_d1 = np.asarray(W_d1, np.float32)
    b_d1 = np.asarray(b_d1, np.float32)
    W_d2 = np.asarray(W_d2, np.float32)
    b_d2 = np.asarray(b_d2, np.float32)
    W_out = np.asarray(W_out, np.float32)

    # host: domain gate MLP + softmax over heads (tiny)
    d1 = np.maximum(domain_label @ W_d1 + b_d1, 0.0)
    d = d1 @ W_d2 + b_d2  # [B, INNER]
    d = d.reshape(B, HEADS, DH)
    e = np.exp(d - d.max(axis=1, keepdims=True))
    gate = (e / e.sum(axis=1, keepdims=True)).reshape(B, INNER).astype(np.float32)

    ones = np.ones((128, 64), BF)
    ones32 = np.ones((1, 64), np.float32)
    in_maps = []
    for c in range(NCORES):
        b, g = c // HG, c % HG
        sl = slice(g * F, (g + 1) * F)
        wq = W_qkv[:, :INNER][:, sl]
        wk = W_qkv[:, INNER : 2 * INNER][:, sl]
        wv = W_qkv[:, 2 * INNER :][:, sl] * gate[b, sl][None, :]
        wall = np.ascontiguousarray(
            np.concatenate([wq, wk, wv], axis=1).astype(BF)
        )
        in_maps.append(
            {
                "xT": np.ascontiguousarray(x[b].T.astype(BF)),
                "wall": wall,
                "wo": np.ascontiguousarray(W_out[sl, :].astype(BF)),
                "ones": ones,
                "ones32": ones32,
            }
        )
    return in_maps


def _run(in_maps, trace=False, tmpdir=None):
    nc = _get_nc()
    return run_bass_kernel_spmd(
        nc, in_maps, list(range(NCORES)), trace=trace, tmpdir=tmpdir
    )


def _assemble(results, b_out):
    b_out = np.asarray(b_out, np.float32)
    out = np.empty((B, N, D), np.float32)
    for b in range(B):
        out[b] = results[HG * b]["part"] + results[HG * b + 1]["part"] + b_out
    return out


def kernel(x, domain_label, W_qkv, W_d1, b_d1, W_d2, b_d2, W_out, b_out):
    in_maps = _prepare_in_maps(
        x, domain_label, W_qkv, W_d1, b_d1, W_d2, b_d2, W_out, b_out
    )
    res = _run(in_maps, trace=False)
    return _assemble(res.results, b_out)
